# revision 53
# baseline (speedup 1.0000x reference)
"""Bass/Trainium2 kernel for naive causal multi-head attention (v8).

Problem: B=4, S=2048, E=1024, H=16, DH=64 (fp32 in/out).

Sharding (8 NeuronCores): core c handles batch b = c//2 and head group
g = c%2 (heads 8g..8g+7).  Each core computes its 8 heads' attention for
its batch plus the partial out-projection through its 512 columns of the
concat dim; the host sums the two partial outputs per batch.

Key structure (all matmuls bf16, PSUM fp32):
  - host pre-transposes/casts x -> xT bf16 and all weights bf16; the
    device streams xT straight into SBUF (no on-device transpose).
  - per head: C1 projects q|k -> qk [128, S]; C2 walks query tiles,
    scores (k-tile^T q) -> exp on ACT -> causal mask on DVE -> PV
    accumulation with a ones-row producing the softmax sums for free.
  - C2 is software-pipelined (scores one group ahead of PV) and the PE
    is kept busy during exp latency by interleaving the NEXT head's C1
    matmuls (or phase-D out-projection matmuls on the last head) as
    filler between attention groups: ACT's exp throughput (~970ns/grp)
    is slower than the 4 attention matmuls (~864ns/grp), so without
    filler the PE would stall every group.
  - softmax normalization is deferred one query-tile: PSUM -> posb,
    reciprocal of sums row, DRAM-bounce broadcast, multiply into cT.
  - out-projection writes bf16; host sums the two partials in f32.
"""

import os

import numpy as np
import ml_dtypes

import concourse.bacc as bacc
import concourse.bass as bass
import concourse.mybir as mybir
from concourse.tile import TileContext
from concourse.bass_utils import run_bass_kernel_spmd

F32 = mybir.dt.float32
BF16 = mybir.dt.bfloat16
EXP = mybir.ActivationFunctionType.Exp

N_CORES = 8
PUMP = int(os.environ.get("K_PUMP", "2"))   # filler units per group slot


def build_nc(S=2048, E=1024, HPC=8, DH=64):
    NQ = 512                      # query-tile width
    nst = S // 128                # token/key tiles
    nec = E // 128                # e-chunks (contraction tiles)
    nqt = S // NQ                 # query tiles
    HD = HPC * DH                 # local concat width (512)
    ncc = HD // 128               # concat chunks (4)

    nc = bacc.Bacc("TRN2", target_bir_lowering=False, debug=False,
                   num_devices=N_CORES)

    xt = nc.dram_tensor("xt", [128, nec, S], BF16, kind="ExternalInput")
    wqkt = nc.dram_tensor("wqkt", [128, HPC, nec, 2 * DH], BF16,
                          kind="ExternalInput")
    wvt = nc.dram_tensor("wvt", [128, nec, HD], BF16, kind="ExternalInput")
    wot = nc.dram_tensor("wot", [128, ncc, E], BF16, kind="ExternalInput")
    mkt = nc.dram_tensor("mkt", [128, 2, 128], BF16, kind="ExternalInput")
    out = nc.dram_tensor("out", [S, E], BF16, kind="ExternalOutput")

    with TileContext(nc) as tc:
        with (
            tc.tile_pool(name="persist", bufs=1) as persist,
            tc.tile_pool(name="qkp", bufs=2) as qkp,
            tc.tile_pool(name="ktp", bufs=2) as ktp,
            tc.tile_pool(name="ptp", bufs=3) as ptp,
            tc.tile_pool(name="recp", bufs=4) as recp,
            tc.tile_pool(name="bcp", bufs=2) as bcp,
            tc.tile_pool(name="outp", bufs=2) as outp,
            tc.tile_pool(name="dramp", bufs=1, space="DRAM") as dramp,
            tc.tile_pool(name="ps_big", bufs=2, space="PSUM") as ps_big,
            tc.tile_pool(name="ps_c1", bufs=1, space="PSUM") as ps_c1,
            tc.tile_pool(name="ps_po", bufs=2, space="PSUM") as ps_po,
        ):
            # ---- persistent SBUF ----
            xT = persist.tile([128, nec, S], BF16)
            wqk = persist.tile([128, HPC, nec, 2 * DH], BF16)
            wv = persist.tile([128, nec, HD], BF16)
            wo = persist.tile([128, ncc, E], BF16)
            vS = persist.tile([128, nst, HPC * (DH + 1)], BF16)
            cT = persist.tile([128, ncc, S], BF16)
            mk = persist.tile([128, 2, 128], BF16)

            nc.vector.memset(vS, 1.0)
            nc.sync.dma_start(out=wv[:, 0:nec // 2], in_=wvt[:, 0:nec // 2, :])
            nc.sync.dma_start(out=wv[:, nec // 2:], in_=wvt[:, nec // 2:, :])
            nc.sync.dma_start(out=mk, in_=mkt[:, :])

            # ---- phase A: stream xT (host pre-transposed), v projection ----
            for jg in range(nqt):
                for ec in range(nec):
                    nc.sync.dma_start(
                        out=xT[:, ec, jg * 512:(jg + 1) * 512],
                        in_=xt[:, ec, jg * 512:(jg + 1) * 512])
                if jg == 0:
                    # q|k / out-proj weights: needed only from C1 onward,
                    # keep them behind the first x chunks in the queues
                    nc.sync.dma_start(out=wqk, in_=wqkt[:, :, :, :])
                    nc.sync.dma_start(out=wo, in_=wot[:, :, :])
                    # preload the ACT exp table during phase A so the
                    # first C2 group doesn't eat the 1.3us table load
                    warm = recp.tile([1, 8], BF16, tag="warm")
                    nc.scalar.activation(out=warm, in_=mk[0:1, 0, 0:8],
                                         func=EXP, scale=0.125)
                for st in range(4 * jg, 4 * jg + 4):
                    pv = ps_po.tile([128, HD], F32, tag="po")
                    for ec in range(nec):
                        nc.tensor.matmul(
                            pv, lhsT=xT[:, ec, st * 128:(st + 1) * 128],
                            rhs=wv[:, ec], start=(ec == 0),
                            stop=(ec == nec - 1))
                    nc.vector.tensor_copy(
                        out=vS[:, st].rearrange("p (h m) -> p h m",
                                                m=DH + 1)[:, :, 0:DH],
                        in_=pv.rearrange("p (h m) -> p h m", m=DH))

            # ---- phase C/D machinery ----
            qk_t = [None] * HPC
            kt_t = [None] * HPC
            filler_q = []           # generators emitting sem-free PE work
            pending = []            # deferred normalizations: (po, h, qt)

            def pump(n):
                done = 0
                while done < n and filler_q:
                    try:
                        next(filler_q[0])
                        done += 1
                    except StopIteration:
                        filler_q.pop(0)

            def drain():
                while filler_q:
                    try:
                        next(filler_q[0])
                    except StopIteration:
                        filler_q.pop(0)

            def c1_gen(h):
                """q|k projection for head h -> qk_t[h], kt_t[h]."""
                qk = qkp.tile([128, S], BF16, tag="qk", name=f"qk{h}")
                kt_sb = ktp.tile([64, S], BF16, tag="kt", name=f"kt{h}")
                qk_t[h], kt_t[h] = qk, kt_sb
                for sc in range(S // 1024):
                    pqk = ps_c1.tile([128, 1024], F32, tag="c1",
                                     name=f"pqk{h}_{sc}")
                    for ec in range(nec):
                        for hf in range(2):
                            nc.tensor.matmul(
                                pqk[:, hf * 512:(hf + 1) * 512],
                                lhsT=wqk[:, h, ec],
                                rhs=xT[:, ec, sc * 1024 + hf * 512:
                                       sc * 1024 + (hf + 1) * 512],
                                start=(ec == 0), stop=(ec == nec - 1),
                                skip_group_check=True)
                        yield
                    nc.vector.tensor_copy(
                        out=qk[:, sc * 1024:(sc + 1) * 1024], in_=pqk)
                    nc.vector.tensor_copy(
                        out=kt_sb[:, sc * 1024:(sc + 1) * 1024],
                        in_=qk[64:128, sc * 1024:(sc + 1) * 1024])
                    yield

            def d_gen(qt):
                """Out-projection for token tiles 4qt..4qt+3 (bf16 out)."""
                for st in range(4 * qt, 4 * qt + 4):
                    pd = ps_c1.tile([128, 1024], F32, tag="c1",
                                    name=f"pd{st}")
                    for c in range(ncc):
                        for hf in range(2):
                            nc.tensor.matmul(
                                pd[:, hf * 512:(hf + 1) * 512],
                                lhsT=cT[:, c, st * 128:(st + 1) * 128],
                                rhs=wo[:, c, hf * 512:(hf + 1) * 512],
                                start=(c == 0), stop=(c == ncc - 1),
                                skip_group_check=True)
                        yield
                    osb = outp.tile([128, E], BF16, tag="osb",
                                    name=f"osb{st}")
                    # split the evacuation across DVE and ACT so the
                    # single-buffered pd bank frees up twice as fast
                    nc.vector.tensor_copy(out=osb[:, 0:512],
                                          in_=pd[:, 0:512])
                    nc.scalar.copy(out=osb[:, 512:1024], in_=pd[:, 512:1024])
                    nc.sync.dma_start(out=out[st * 128:(st + 1) * 128, :],
                                      in_=osb)
                    yield

            def emit_norm():
                """Normalize the oldest pending (h, qt)'s attention cols."""
                po_p, h_p, qt_p = pending.pop(0)
                posb = bcp.tile([DH + 1, NQ], F32, tag="posb")
                nc.vector.tensor_copy(out=posb, in_=po_p[0:DH + 1, :])
                # custom-DVE ops can't partition-shift: stage the sums row
                # to partition 0 with a plain copy first
                sums0 = recp.tile([1, NQ], F32, tag="sums0")
                nc.vector.tensor_copy(out=sums0, in_=posb[DH:DH + 1, :])
                rec = recp.tile([1, NQ], F32, tag="rec")
                nc.vector.reciprocal_approx_fast(out=rec, in_=sums0)
                # broadcast across 64 partitions via DRAM bounce
                recd = dramp.tile([1, NQ], F32, tag="recd", bufs=4)
                nc.sync.dma_start(out=recd, in_=rec)
                bc = bcp.tile([64, NQ], F32, tag="bc")
                nc.sync.dma_start(
                    out=bc,
                    in_=bass.AP(tensor=recd.tensor, offset=recd.offset,
                                ap=[[0, 64]] + list(recd.ap[1:])))
                nc.vector.tensor_mul(
                    cT[64 * (h_p % 2):64 * (h_p % 2) + 64, h_p // 2,
                       qt_p * NQ:(qt_p + 1) * NQ],
                    posb[0:DH, :], bc)
                return h_p, qt_p

            # ---- phase C: per head; C1(h+1)/D matmuls fill exp latency ----
            filler_q.append(c1_gen(0))
            drain()
            for h in range(HPC):
                qk, kt_sb = qk_t[h], kt_t[h]
                if h + 1 < HPC:
                    filler_q.append(c1_gen(h + 1))
                for qt in range(nqt):
                    ngrp = 2 * qt + 2
                    po = ps_po.tile([DH + 1, NQ], F32, tag="po")
                    pts = {}

                    def blk(g, kk):
                        """Column base & query offset for block (g, kk).

                        Diagonal kk=1 blocks are packed at base 384+n0 so
                        the group's two causal triangles sit exactly 512
                        apart (one strided mask multiply covers both).
                        """
                        kt = 2 * g + kk
                        d = kt - 4 * qt
                        n0 = 128 * d if d > 0 else 0
                        base = (384 + n0) if (d > 0 and kk == 1) \
                            else kk * 512 + n0
                        return kt, n0, base

                    def emit_scores(g):
                        ps2 = ps_big.tile([128, 1024], F32, tag="big")
                        for kk in range(2):
                            kt, n0, base = blk(g, kk)
                            nc.tensor.matmul(
                                ps2[:, base:base + NQ - n0],
                                lhsT=kt_sb[:, kt * 128:(kt + 1) * 128],
                                rhs=qk[0:64, qt * NQ + n0:(qt + 1) * NQ],
                                start=True, stop=True, skip_group_check=True)
                        pt = ptp.tile([128, 1024], BF16, tag="pt")
                        if g == 2 * qt:          # diag A: [0:512]+[512:896]
                            nc.scalar.activation(out=pt[:, 0:896],
                                                 in_=ps2[:, 0:896],
                                                 func=EXP, scale=0.125)
                        elif g == 2 * qt + 1:    # diag B: [256:512]+[768:896]
                            nc.scalar.activation(out=pt[:, 256:512],
                                                 in_=ps2[:, 256:512],
                                                 func=EXP, scale=0.125)
                            nc.scalar.activation(out=pt[:, 768:896],
                                                 in_=ps2[:, 768:896],
                                                 func=EXP, scale=0.125)
                        else:
                            nc.scalar.activation(out=pt, in_=ps2,
                                                 func=EXP, scale=0.125)
                        if g >= 2 * qt:
                            # both triangles of this diagonal group in one
                            # strided multiply: cols {t0, t0+512}
                            t0 = 0 if g == 2 * qt else 256
                            ptk = pt.rearrange("p (k c) -> p k c", c=NQ)
                            nc.vector.tensor_mul(
                                ptk[:, :, t0:t0 + 128],
                                ptk[:, :, t0:t0 + 128], mk[:, :, :])
                        pts[g] = pt

                    def emit_pv(g):
                        pt = pts.pop(g)
                        for kk in range(2):
                            kt, n0, base = blk(g, kk)
                            nc.tensor.matmul(
                                po[:, n0:NQ],
                                lhsT=vS[:, kt, h * (DH + 1):(h + 1) * (DH + 1)],
                                rhs=pt[:, base:base + NQ - n0],
                                start=(g == 0 and kk == 0),
                                stop=(g == ngrp - 1 and kk == 1),
                                skip_group_check=True)

                    rate = PUMP + (1 if h == HPC - 1 else 0)
                    emit_scores(0)
                    emit_scores(1)
                    if pending:
                        h_p, qt_p = emit_norm()
                        if h_p == HPC - 1:      # head-7 cT cols ready:
                            filler_q.append(d_gen(qt_p))  # out-proj filler
                    pump(rate)
                    for g in range(ngrp):
                        # PV first: pt(g)'s reads must be emitted before
                        # the pool hands pt(g)'s buffer to scores(g+2)
                        emit_pv(g)
                        pump(rate)
                        if g + 2 < ngrp:
                            emit_scores(g + 2)
                    pending.append((po, h, qt))
                drain()   # finish C1(h+1)/D stragglers before next head

            h_p, qt_p = emit_norm()  # last (h=7, qt=3)
            filler_q.append(d_gen(qt_p))
            drain()

    nc.finalize()
    return nc


def _host_prep(x, Wq, Wk, Wv, Wo, HPC=8, DH=64):
    """Build the 8 per-core input maps (everything bf16)."""
    B, S, E = x.shape
    nec = E // 128
    HD = HPC * DH
    bf = ml_dtypes.bfloat16
    j = np.arange(128)[:, None]
    m = np.arange(128)[None, :]
    tri = (j <= m).astype(np.float32).astype(bf)
    mkt = np.ascontiguousarray(np.stack([tri, tri], axis=1))  # [128,2,128]
    in_maps = []
    # x[b] -> [128, nec, S] bf16 (E on partitions, host-transposed)
    xtl = [np.ascontiguousarray(
        x[b].T.reshape(nec, 128, S).transpose(1, 0, 2).astype(bf))
        for b in range(B)]
    for c in range(N_CORES):
        b, g = c // 2, c % 2
        hs = slice(HPC * g, HPC * g + HPC)
        wqk = np.concatenate([Wq[hs], Wk[hs]], axis=1)          # [HPC,128,E]
        wqk = wqk.transpose(2, 0, 1).reshape(nec, 128, HPC, 2 * DH)
        wqkt = np.ascontiguousarray(wqk.transpose(1, 2, 0, 3).astype(bf))
        wvt = Wv[hs].transpose(2, 0, 1).reshape(nec, 128, HD)
        wvt = np.ascontiguousarray(wvt.transpose(1, 0, 2).astype(bf))
        wot = np.ascontiguousarray(Wo[:, HD * g:HD * (g + 1)].T)  # [HD, E]
        wot = np.ascontiguousarray(
            wot.reshape(HD // 128, 128, E).transpose(1, 0, 2).astype(bf))
        in_maps.append({
            "xt": xtl[b], "wqkt": wqkt, "wvt": wvt, "wot": wot, "mkt": mkt,
        })
    return in_maps


_NC_CACHE = {}


def kernel(x, Wq, Wk, Wv, Wo):
    x = np.asarray(x, dtype=np.float32)
    Wq = np.asarray(Wq, dtype=np.float32)
    Wk = np.asarray(Wk, dtype=np.float32)
    Wv = np.asarray(Wv, dtype=np.float32)
    Wo = np.asarray(Wo, dtype=np.float32)
    B, S, E = x.shape
    H, DH, _ = Wq.shape
    HPC = H // 2

    key = (S, E, HPC, DH)
    if key not in _NC_CACHE:
        _NC_CACHE[key] = build_nc(S=S, E=E, HPC=HPC, DH=DH)
    nc = _NC_CACHE[key]

    in_maps = _host_prep(x, Wq, Wk, Wv, Wo, HPC=HPC, DH=DH)
    res = run_bass_kernel_spmd(nc, in_maps, core_ids=list(range(N_CORES)))
    kernel.last_results = res

    out = np.empty((B, S, E), dtype=np.float32)
    for b in range(B):
        out[b] = (res.results[2 * b]["out"].astype(np.float32)
                  + res.results[2 * b + 1]["out"].astype(np.float32))
    return out


# revision 54
# speedup vs baseline: 1.0256x; 1.0256x over previous
"""Bass/Trainium2 kernel for naive causal multi-head attention (v8).

Problem: B=4, S=2048, E=1024, H=16, DH=64 (fp32 in/out).

Sharding (8 NeuronCores): core c handles batch b = c//2 and head group
g = c%2 (heads 8g..8g+7).  Each core computes its 8 heads' attention for
its batch plus the partial out-projection through its 512 columns of the
concat dim; the host sums the two partial outputs per batch.

Key structure (all matmuls bf16, PSUM fp32):
  - host pre-transposes/casts x -> xT bf16 and all weights bf16; the
    device streams xT straight into SBUF (no on-device transpose).
  - per head: C1 projects q|k -> qk [128, S]; C2 walks query tiles,
    scores (k-tile^T q) -> exp on ACT -> causal mask on DVE -> PV
    accumulation with a ones-row producing the softmax sums for free.
  - C2 is software-pipelined (scores one group ahead of PV) and the PE
    is kept busy during exp latency by interleaving the NEXT head's C1
    matmuls (or phase-D out-projection matmuls on the last head) as
    filler between attention groups: ACT's exp throughput (~970ns/grp)
    is slower than the 4 attention matmuls (~864ns/grp), so without
    filler the PE would stall every group.
  - softmax normalization is deferred one query-tile: PSUM -> posb,
    reciprocal of sums row, DRAM-bounce broadcast, multiply into cT.
  - out-projection writes bf16; host sums the two partials in f32.
"""

import os

import numpy as np
import ml_dtypes

import concourse.bacc as bacc
import concourse.bass as bass
import concourse.mybir as mybir
from concourse.tile import TileContext
from concourse.bass_utils import run_bass_kernel_spmd

F32 = mybir.dt.float32
BF16 = mybir.dt.bfloat16
EXP = mybir.ActivationFunctionType.Exp

N_CORES = 8
PUMP = int(os.environ.get("K_PUMP", "2"))   # filler units per group slot


def build_nc(S=2048, E=1024, HPC=8, DH=64):
    NQ = 512                      # query-tile width
    nst = S // 128                # token/key tiles
    nec = E // 128                # e-chunks (contraction tiles)
    nqt = S // NQ                 # query tiles
    HD = HPC * DH                 # local concat width (512)
    ncc = HD // 128               # concat chunks (4)

    nc = bacc.Bacc("TRN2", target_bir_lowering=False, debug=False,
                   num_devices=N_CORES)

    xt = nc.dram_tensor("xt", [128, nec, S], BF16, kind="ExternalInput")
    wqkt = nc.dram_tensor("wqkt", [128, HPC, nec, 2 * DH], BF16,
                          kind="ExternalInput")
    wvt = nc.dram_tensor("wvt", [128, nec, HD], BF16, kind="ExternalInput")
    wot = nc.dram_tensor("wot", [128, ncc, E], BF16, kind="ExternalInput")
    mkt = nc.dram_tensor("mkt", [128, 2, 128], BF16, kind="ExternalInput")
    out = nc.dram_tensor("out", [S, E], BF16, kind="ExternalOutput")

    with TileContext(nc) as tc:
        with (
            tc.tile_pool(name="persist", bufs=1) as persist,
            tc.tile_pool(name="qkp", bufs=2) as qkp,
            tc.tile_pool(name="ktp", bufs=2) as ktp,
            tc.tile_pool(name="ptp", bufs=3) as ptp,
            tc.tile_pool(name="recp", bufs=4) as recp,
            tc.tile_pool(name="bcp", bufs=2) as bcp,
            tc.tile_pool(name="outp", bufs=2) as outp,
            tc.tile_pool(name="dramp", bufs=1, space="DRAM") as dramp,
            tc.tile_pool(name="ps_big", bufs=3, space="PSUM") as ps_big,
            tc.tile_pool(name="ps_po", bufs=2, space="PSUM") as ps_po,
        ):
            # ---- persistent SBUF ----
            xT = persist.tile([128, nec, S], BF16)
            wqk = persist.tile([128, HPC, nec, 2 * DH], BF16)
            wv = persist.tile([128, nec, HD], BF16)
            wo = persist.tile([128, ncc, E], BF16)
            vS = persist.tile([128, nst, HPC * (DH + 1)], BF16)
            cT = persist.tile([128, ncc, S], BF16)
            mk = persist.tile([128, 2, 128], BF16)

            nc.vector.memset(vS, 1.0)
            nc.sync.dma_start(out=wv[:, 0:nec // 2], in_=wvt[:, 0:nec // 2, :])
            nc.sync.dma_start(out=wv[:, nec // 2:], in_=wvt[:, nec // 2:, :])
            nc.sync.dma_start(out=mk, in_=mkt[:, :])

            # ---- phase A: stream xT (host pre-transposed), v projection ----
            for jg in range(nqt):
                for ec in range(nec):
                    nc.sync.dma_start(
                        out=xT[:, ec, jg * 512:(jg + 1) * 512],
                        in_=xt[:, ec, jg * 512:(jg + 1) * 512])
                if jg == 0:
                    # q|k / out-proj weights: needed only from C1 onward,
                    # keep them behind the first x chunks in the queues
                    nc.sync.dma_start(out=wqk, in_=wqkt[:, :, :, :])
                    nc.sync.dma_start(out=wo, in_=wot[:, :, :])
                    # preload the ACT exp table during phase A so the
                    # first C2 group doesn't eat the 1.3us table load
                    warm = recp.tile([1, 8], BF16, tag="warm")
                    nc.scalar.activation(out=warm, in_=mk[0:1, 0, 0:8],
                                         func=EXP, scale=0.125)
                for st in range(4 * jg, 4 * jg + 4):
                    pv = ps_po.tile([128, HD], F32, tag="po")
                    for ec in range(nec):
                        nc.tensor.matmul(
                            pv, lhsT=xT[:, ec, st * 128:(st + 1) * 128],
                            rhs=wv[:, ec], start=(ec == 0),
                            stop=(ec == nec - 1))
                    nc.vector.tensor_copy(
                        out=vS[:, st].rearrange("p (h m) -> p h m",
                                                m=DH + 1)[:, :, 0:DH],
                        in_=pv.rearrange("p (h m) -> p h m", m=DH))

            # ---- phase C/D machinery ----
            qk_t = [None] * HPC
            kt_t = [None] * HPC
            filler_q = []           # generators emitting sem-free PE work
            pending = []            # deferred normalizations: (po, h, qt)

            def pump(n):
                done = 0
                while done < n and filler_q:
                    try:
                        next(filler_q[0])
                        done += 1
                    except StopIteration:
                        filler_q.pop(0)

            def drain():
                while filler_q:
                    try:
                        next(filler_q[0])
                    except StopIteration:
                        filler_q.pop(0)

            def c1_gen(h):
                """q|k projection for head h -> qk_t[h], kt_t[h]."""
                qk = qkp.tile([128, S], BF16, tag="qk", name=f"qk{h}")
                kt_sb = ktp.tile([64, S], BF16, tag="kt", name=f"kt{h}")
                qk_t[h], kt_t[h] = qk, kt_sb
                for sc in range(S // 1024):
                    pqk = ps_big.tile([128, 1024], F32, tag="big",
                                      name=f"pqk{h}_{sc}")
                    for ec in range(nec):
                        for hf in range(2):
                            nc.tensor.matmul(
                                pqk[:, hf * 512:(hf + 1) * 512],
                                lhsT=wqk[:, h, ec],
                                rhs=xT[:, ec, sc * 1024 + hf * 512:
                                       sc * 1024 + (hf + 1) * 512],
                                start=(ec == 0), stop=(ec == nec - 1),
                                skip_group_check=True)
                        yield
                    nc.vector.tensor_copy(
                        out=qk[:, sc * 1024:(sc + 1) * 1024], in_=pqk)
                    nc.vector.tensor_copy(
                        out=kt_sb[:, sc * 1024:(sc + 1) * 1024],
                        in_=qk[64:128, sc * 1024:(sc + 1) * 1024])
                    yield

            def d_gen(qt):
                """Out-projection for token tiles 4qt..4qt+3 (bf16 out)."""
                for st in range(4 * qt, 4 * qt + 4):
                    pd = ps_big.tile([128, 1024], F32, tag="big",
                                     name=f"pd{st}")
                    for c in range(ncc):
                        for hf in range(2):
                            nc.tensor.matmul(
                                pd[:, hf * 512:(hf + 1) * 512],
                                lhsT=cT[:, c, st * 128:(st + 1) * 128],
                                rhs=wo[:, c, hf * 512:(hf + 1) * 512],
                                start=(c == 0), stop=(c == ncc - 1),
                                skip_group_check=True)
                        yield
                    osb = outp.tile([128, E], BF16, tag="osb",
                                    name=f"osb{st}")
                    # split the evacuation across DVE and ACT so the
                    # single-buffered pd bank frees up twice as fast
                    nc.vector.tensor_copy(out=osb[:, 0:512],
                                          in_=pd[:, 0:512])
                    nc.scalar.copy(out=osb[:, 512:1024], in_=pd[:, 512:1024])
                    nc.sync.dma_start(out=out[st * 128:(st + 1) * 128, :],
                                      in_=osb)
                    yield

            def emit_norm():
                """Normalize the oldest pending (h, qt)'s attention cols."""
                po_p, h_p, qt_p = pending.pop(0)
                posb = bcp.tile([DH + 1, NQ], F32, tag="posb")
                nc.vector.tensor_copy(out=posb, in_=po_p[0:DH + 1, :])
                # custom-DVE ops can't partition-shift: stage the sums row
                # to partition 0 with a plain copy first
                sums0 = recp.tile([1, NQ], F32, tag="sums0")
                nc.vector.tensor_copy(out=sums0, in_=posb[DH:DH + 1, :])
                rec = recp.tile([1, NQ], F32, tag="rec")
                nc.vector.reciprocal_approx_fast(out=rec, in_=sums0)
                # broadcast across 64 partitions via DRAM bounce
                recd = dramp.tile([1, NQ], F32, tag="recd", bufs=4)
                nc.sync.dma_start(out=recd, in_=rec)
                bc = bcp.tile([64, NQ], F32, tag="bc")
                nc.sync.dma_start(
                    out=bc,
                    in_=bass.AP(tensor=recd.tensor, offset=recd.offset,
                                ap=[[0, 64]] + list(recd.ap[1:])))
                nc.vector.tensor_mul(
                    cT[64 * (h_p % 2):64 * (h_p % 2) + 64, h_p // 2,
                       qt_p * NQ:(qt_p + 1) * NQ],
                    posb[0:DH, :], bc)
                return h_p, qt_p

            # ---- phase C: per head; C1(h+1)/D matmuls fill exp latency ----
            for h in range(HPC):
                filler_q.append(c1_gen(h))
                drain()           # C1 as one contiguous full-clock stream
                qk, kt_sb = qk_t[h], kt_t[h]
                for qt in range(nqt):
                    ngrp = 2 * qt + 2
                    po = ps_po.tile([DH + 1, NQ], F32, tag="po")
                    pts = {}

                    def blk(g, kk):
                        """Column base & query offset for block (g, kk).

                        Diagonal kk=1 blocks are packed at base 384+n0 so
                        the group's two causal triangles sit exactly 512
                        apart (one strided mask multiply covers both).
                        """
                        kt = 2 * g + kk
                        d = kt - 4 * qt
                        n0 = 128 * d if d > 0 else 0
                        base = (384 + n0) if (d > 0 and kk == 1) \
                            else kk * 512 + n0
                        return kt, n0, base

                    def emit_scores(g):
                        ps2 = ps_big.tile([128, 1024], F32, tag="big")
                        for kk in range(2):
                            kt, n0, base = blk(g, kk)
                            nc.tensor.matmul(
                                ps2[:, base:base + NQ - n0],
                                lhsT=kt_sb[:, kt * 128:(kt + 1) * 128],
                                rhs=qk[0:64, qt * NQ + n0:(qt + 1) * NQ],
                                start=True, stop=True, skip_group_check=True)
                        pt = ptp.tile([128, 1024], BF16, tag="pt")
                        if g == 2 * qt:          # diag A: [0:512]+[512:896]
                            nc.scalar.activation(out=pt[:, 0:896],
                                                 in_=ps2[:, 0:896],
                                                 func=EXP, scale=0.125)
                        elif g == 2 * qt + 1:    # diag B: [256:512]+[768:896]
                            nc.scalar.activation(out=pt[:, 256:512],
                                                 in_=ps2[:, 256:512],
                                                 func=EXP, scale=0.125)
                            nc.scalar.activation(out=pt[:, 768:896],
                                                 in_=ps2[:, 768:896],
                                                 func=EXP, scale=0.125)
                        else:
                            nc.scalar.activation(out=pt, in_=ps2,
                                                 func=EXP, scale=0.125)
                        if g >= 2 * qt:
                            # both triangles of this diagonal group in one
                            # strided multiply: cols {t0, t0+512}
                            t0 = 0 if g == 2 * qt else 256
                            ptk = pt.rearrange("p (k c) -> p k c", c=NQ)
                            nc.vector.tensor_mul(
                                ptk[:, :, t0:t0 + 128],
                                ptk[:, :, t0:t0 + 128], mk[:, :, :])
                        pts[g] = pt

                    def emit_pv(g):
                        pt = pts.pop(g)
                        for kk in range(2):
                            kt, n0, base = blk(g, kk)
                            nc.tensor.matmul(
                                po[:, n0:NQ],
                                lhsT=vS[:, kt, h * (DH + 1):(h + 1) * (DH + 1)],
                                rhs=pt[:, base:base + NQ - n0],
                                start=(g == 0 and kk == 0),
                                stop=(g == ngrp - 1 and kk == 1),
                                skip_group_check=True)

                    rate = PUMP + (1 if h == HPC - 1 else 0)
                    for g in range(min(3, ngrp)):
                        emit_scores(g)
                    if pending:
                        h_p, qt_p = emit_norm()
                        if h_p == HPC - 1:      # head-7 cT cols ready:
                            filler_q.append(d_gen(qt_p))  # out-proj filler
                    pump(rate)
                    for g in range(ngrp):
                        # PV first: pt(g)'s reads must be emitted before
                        # the pool hands pt(g)'s buffer to scores(g+2)
                        emit_pv(g)
                        pump(rate)
                        if g + 3 < ngrp:
                            emit_scores(g + 3)
                    pending.append((po, h, qt))
                drain()   # finish C1(h+1)/D stragglers before next head

            h_p, qt_p = emit_norm()  # last (h=7, qt=3)
            filler_q.append(d_gen(qt_p))
            drain()

    nc.finalize()
    return nc


def _host_prep(x, Wq, Wk, Wv, Wo, HPC=8, DH=64):
    """Build the 8 per-core input maps (everything bf16)."""
    B, S, E = x.shape
    nec = E // 128
    HD = HPC * DH
    bf = ml_dtypes.bfloat16
    j = np.arange(128)[:, None]
    m = np.arange(128)[None, :]
    tri = (j <= m).astype(np.float32).astype(bf)
    mkt = np.ascontiguousarray(np.stack([tri, tri], axis=1))  # [128,2,128]
    in_maps = []
    # x[b] -> [128, nec, S] bf16 (E on partitions, host-transposed)
    xtl = [np.ascontiguousarray(
        x[b].T.reshape(nec, 128, S).transpose(1, 0, 2).astype(bf))
        for b in range(B)]
    for c in range(N_CORES):
        b, g = c // 2, c % 2
        hs = slice(HPC * g, HPC * g + HPC)
        wqk = np.concatenate([Wq[hs], Wk[hs]], axis=1)          # [HPC,128,E]
        wqk = wqk.transpose(2, 0, 1).reshape(nec, 128, HPC, 2 * DH)
        wqkt = np.ascontiguousarray(wqk.transpose(1, 2, 0, 3).astype(bf))
        wvt = Wv[hs].transpose(2, 0, 1).reshape(nec, 128, HD)
        wvt = np.ascontiguousarray(wvt.transpose(1, 0, 2).astype(bf))
        wot = np.ascontiguousarray(Wo[:, HD * g:HD * (g + 1)].T)  # [HD, E]
        wot = np.ascontiguousarray(
            wot.reshape(HD // 128, 128, E).transpose(1, 0, 2).astype(bf))
        in_maps.append({
            "xt": xtl[b], "wqkt": wqkt, "wvt": wvt, "wot": wot, "mkt": mkt,
        })
    return in_maps


_NC_CACHE = {}


def kernel(x, Wq, Wk, Wv, Wo):
    x = np.asarray(x, dtype=np.float32)
    Wq = np.asarray(Wq, dtype=np.float32)
    Wk = np.asarray(Wk, dtype=np.float32)
    Wv = np.asarray(Wv, dtype=np.float32)
    Wo = np.asarray(Wo, dtype=np.float32)
    B, S, E = x.shape
    H, DH, _ = Wq.shape
    HPC = H // 2

    key = (S, E, HPC, DH)
    if key not in _NC_CACHE:
        _NC_CACHE[key] = build_nc(S=S, E=E, HPC=HPC, DH=DH)
    nc = _NC_CACHE[key]

    in_maps = _host_prep(x, Wq, Wk, Wv, Wo, HPC=HPC, DH=DH)
    res = run_bass_kernel_spmd(nc, in_maps, core_ids=list(range(N_CORES)))
    kernel.last_results = res

    out = np.empty((B, S, E), dtype=np.float32)
    for b in range(B):
        out[b] = (res.results[2 * b]["out"].astype(np.float32)
                  + res.results[2 * b + 1]["out"].astype(np.float32))
    return out
